# revision 15
# baseline (speedup 1.0000x reference)
"""2D DWT (db4, circular pad, stride-2) forward on 8 Trainium2 NeuronCores.

Strategy (pure data parallel, 12 images of 512x512 per core):
Both separable filter passes are banded matmuls on the TensorEngine:

  stage 1 (filter along H):  V[w, (hj,a)]   = sum_h  X[h, w] * M[h, (hj,a)]
  stage 2 (filter along W):  out[hj,(wj,b)] = sum_w  V[w, (hj,a)] * M[w, (wj,b)]

M[i, 2j+f] = dec[f][(i-2j)%512] (8 nonzeros per column). Each 128-row
chunk of M has a 134-wide contiguous band of nonzero columns (wrapping
once), so each PSUM accumulation group streams 536 of 512 columns in 5
banded matmuls instead of dense 512-wide chunks.

The 2e-2 rel-err gate leaves room for a single fp16 pass (measured
~8e-4), so vs the 3-term fp16-split baseline this does 1/3 the matmul
work, half the input DMA (x fp16 once) and half the output DMA (out
fp16, upcast on host). All DRAM<->SBUF transfers are host-pre/post-
shuffled to be fully contiguous (one 512KB DMA per image each way), and
M is sent band-compacted (137KB). PSUM->SBUF de-interleave copies are
split between DVE and Act so neither engine bottlenecks; program order
interleaves stage1(img) with stage2(img-1) so the PE never waits on the
V copies.
"""

import sys

sys.path.insert(0, "/opt/trn_rl_repo")

import numpy as np

L = 512
NJ = L // 2  # 256
TAPS = 8
N_CORES = 8
IMGS_PER_CORE = 12  # 32 batch * 3 channels / 8 cores
BW = 134  # nonzero interleaved-column band width per 128-row chunk

_compiled = {}

# Banded matmul slices per accumulation group: (chunk, src0, src1, dst0, dst1)
# src = cols of the compact band tile, dst = cols of the 512-wide PSUM bank.
# Chunk c covers interleaved cols [128c-6, 128c+128) mod 512; c=0 wraps and
# splits in two. Order keeps the tiny 6-wide stream between long ones so its
# weight load hides behind them.
_SLICES = [
    (0, 0, 6, 506, 512),
    (1, 0, BW, 122, 256),
    (0, 6, BW, 0, 128),
    (2, 0, BW, 250, 384),
    (3, 0, BW, 378, 512),
]


def _build_mc(dec: np.ndarray) -> np.ndarray:
    """Compact banded filter matrix: mc[p, c*134 + k] = M[128c+p, (128c-6+k)%512]
    where M[i, 2j+f] = dec[f][(i-2j) % 512] (zero unless (i-2j)%512 < 8)."""
    M = np.zeros((L, L), dtype=np.float32)
    i = np.arange(L)[:, None]
    j = np.arange(NJ)[None, :]
    k = (i - 2 * j) % L
    mask = k < TAPS
    for f in range(2):
        M[:, f::2] = np.where(mask, np.asarray(dec[f])[np.minimum(k, TAPS - 1)], 0.0)
    mc = np.zeros((128, 4 * BW), dtype=np.float16)
    for c in range(4):
        cols = (128 * c - 6 + np.arange(BW)) % L
        mc[:, BW * c : BW * (c + 1)] = M[128 * c : 128 * c + 128, cols]
    return mc


def _build_nc():
    import concourse.bass as bass  # noqa: F401
    import concourse.tile as tile
    from concourse import bacc, mybir

    f32 = mybir.dt.float32
    f16 = mybir.dt.float16
    nc = bacc.Bacc("TRN2", target_bir_lowering=False, debug=False,
                   num_devices=N_CORES)
    x_d = nc.dram_tensor("xc", [IMGS_PER_CORE, 128, 4 * L], f16,
                         kind="ExternalInput")
    mc_d = nc.dram_tensor("mc", [128, 4 * BW], f16, kind="ExternalInput")
    o_d = nc.dram_tensor("out", [IMGS_PER_CORE, 2, 128, 4 * NJ], f16,
                         kind="ExternalOutput")

    with tile.TileContext(nc) as tc:
        with (
            tc.tile_pool(name="mpool", bufs=1) as mpool,
            tc.tile_pool(name="xpool", bufs=6) as xpool,
            tc.tile_pool(name="vpool", bufs=2) as vpool,
            tc.tile_pool(name="opool", bufs=6) as opool,
            tc.tile_pool(name="pvpool", bufs=2, space="PSUM") as pvpool,
            tc.tile_pool(name="popool", bufs=2, space="PSUM") as popool,
        ):
            mct = mpool.tile([128, 4 * BW], f16, tag="mct")
            # ring the small filter-bank DMA from Act, in parallel with SP
            # ringing image 0's chunks, so neither delays the other
            nc.scalar.dma_start(mct[:], mc_d[:])

            def group(psum_bank, stationary_of_chunk):
                """One 512-col accumulation group: 5 banded matmuls."""
                for n, (c, s0, s1, d0, d1) in enumerate(_SLICES):
                    nc.tensor.matmul(
                        psum_bank[:, d0:d1],
                        stationary_of_chunk(c),
                        mct[:, BW * c + s0 : BW * c + s1],
                        start=(n == 0),
                        stop=(n == len(_SLICES) - 1),
                    )

            vts = [None, None]  # vt of img, img-1

            def stage1(img, xt):
                vt = vpool.tile([128, 4 * L], f16, tag="vt")
                for pair in range(2):
                    pv = pvpool.tile([128, 2 * L], f32, tag="pv")  # 2 banks
                    for wi in range(2):
                        wc = 2 * pair + wi
                        group(
                            pv[:, L * wi : L * wi + L],
                            lambda c: xt[:, L * c + 128 * wc : L * c + 128 * wc + 128],
                        )
                    # de-interleave V: dst (wi, a, j) <- src[512wi + 2j + a];
                    # halves on DVE + Act in parallel so the PSUM pair frees
                    # in ~1us and the PE never stalls (keeps its DVFS ramp)
                    for wi, eng in ((0, nc.vector.tensor_copy), (1, nc.scalar.copy)):
                        eng(
                            vt[:, 2 * L * pair + L * wi : 2 * L * pair + L * (wi + 1)]
                            .rearrange("p (a j) -> p a j", a=2),
                            pv[:, L * wi : L * (wi + 1)].rearrange(
                                "p (j a) -> p a j", j=NJ, a=2
                            ),
                        )
                return vt

            def stage2(img, vt):
                ot = opool.tile([128, 8 * NJ], f16, tag="ot")
                for hjc in range(2):
                    po = popool.tile([128, 2 * L], f32, tag="po")
                    for a in range(2):
                        off = NJ * a + 128 * hjc
                        group(
                            po[:, L * a : L * a + L],
                            lambda c: vt[:, L * c + off : L * c + off + 128],
                        )
                    # subbands: dst (b, a, wj) <- src[p, 512a + 2wj + b];
                    # b-halves split across DVE + Act
                    for b, eng in ((0, nc.vector.tensor_copy), (1, nc.scalar.copy)):
                        eng(
                            ot[:, 4 * NJ * hjc + 2 * NJ * b : 4 * NJ * hjc + 2 * NJ * (b + 1)]
                            .rearrange("p (a w) -> p a w", a=2),
                            po[:, b : 2 * L : 2].rearrange(
                                "p (a w) -> p a w", a=2
                            ),
                        )
                # output rings on SP: Act is near its throughput budget
                # with the copies, and input prefetch has enough runway that
                # head-of-line waits on SP are harmless. The last image rings
                # per-half so its first half transfers while the second
                # half's copy still runs (shorter tail).
                if img == IMGS_PER_CORE - 1:
                    for hjc in range(2):
                        nc.sync.dma_start(
                            o_d[img, hjc],
                            ot[:, 4 * NJ * hjc : 4 * NJ * (hjc + 1)],
                        )
                else:
                    nc.sync.dma_start(
                        o_d[img].rearrange("h p c -> p h c"),
                        ot[:].rearrange("p (h c) -> p h c", h=2),
                    )

            # issue ALL input rings up front on SP: each is gated only by
            # its xpool-slot semaphore, and none can queue behind an output
            # ring (whose copy-completion waits would delay late inputs)
            xts = []
            for img in range(IMGS_PER_CORE):
                xt = xpool.tile([128, 4 * L], f16, tag="xt")
                xts.append(xt)
                if img == 0:
                    # split the first image per chunk so its chunk-0
                    # matmuls start ~2us before the full image lands
                    for c in range(4):
                        nc.sync.dma_start(
                            xt[:, L * c : L * (c + 1)],
                            x_d[img][:, L * c : L * (c + 1)],
                        )
                else:
                    nc.sync.dma_start(xt[:], x_d[img])

            # software pipeline: stage1(img) then stage2(img-1), so stage2's
            # dependence on the V copies never stalls the PE.
            for img in range(IMGS_PER_CORE + 1):
                if img < IMGS_PER_CORE:
                    vts[0] = stage1(img, xts[img])
                if img > 0:
                    stage2(img - 1, vts[1])
                vts[1] = vts[0]

    nc.finalize()
    return nc


def _in_maps(x: np.ndarray, dec: np.ndarray) -> list[dict]:
    mc = _build_mc(dec)
    # host pre-shuffle to (img, p, (c, w)): each image is one contiguous
    # 512KB DMA with 4KB per-partition lines
    xc = np.ascontiguousarray(
        x.reshape(96, 4, 128, L).astype(np.float16).transpose(0, 2, 1, 3)
    ).reshape(96, 128, 4 * L)
    return [
        {"xc": xc[IMGS_PER_CORE * c : IMGS_PER_CORE * (c + 1)], "mc": mc}
        for c in range(N_CORES)
    ]


def kernel(x: np.ndarray, dec: np.ndarray) -> np.ndarray:
    from concourse.bass_utils import run_bass_kernel_spmd

    x = np.ascontiguousarray(np.asarray(x, dtype=np.float32))
    dec = np.asarray(dec, dtype=np.float32)
    B, C, H, W = x.shape
    assert (B, C, H, W) == (32, 3, 512, 512) and dec.shape == (2, 8)

    if "nc" not in _compiled:
        _compiled["nc"] = _build_nc()
    nc = _compiled["nc"]

    in_maps = _in_maps(x, dec)
    res = run_bass_kernel_spmd(nc, in_maps, list(range(N_CORES))).results
    o = np.concatenate([r["out"] for r in res], axis=0)  # (96, 2, 128, 1024) f16
    # o[i, hjc, p, (s, wj)] -> out[i, s, 128*hjc + p, wj]
    o = o.reshape(96, 2, 128, 4, NJ).transpose(0, 3, 1, 2, 4)
    return np.ascontiguousarray(o, dtype=np.float32).reshape(B, C * 4, H // 2, W // 2)


# revision 16
# speedup vs baseline: 1.1458x; 1.1458x over previous
"""2D DWT (db4, circular pad, stride-2) forward on 8 Trainium2 NeuronCores.

Strategy (pure data parallel, 12 images of 512x512 per core):
Both separable filter passes are banded matmuls on the TensorEngine:

  stage 1 (filter along H):  V[w, (hj,a)]   = sum_h  X[h, w] * M[h, (hj,a)]
  stage 2 (filter along W):  out[hj,(wj,b)] = sum_w  V[w, (hj,a)] * M[w, (wj,b)]

M[i, 2j+f] = dec[f][(i-2j)%512] (8 nonzeros per column). Each 128-row
chunk of M has a 134-wide contiguous band of nonzero columns (wrapping
once), so each PSUM accumulation group streams 536 of 512 columns in 5
banded matmuls instead of dense 512-wide chunks.

The 2e-2 rel-err gate leaves room for a single fp16 pass (measured
~8e-4), so vs the 3-term fp16-split baseline this does 1/3 the matmul
work, half the input DMA (x fp16 once) and half the output DMA (out
fp16, upcast on host). All DRAM<->SBUF transfers are host-pre/post-
shuffled to be fully contiguous (one 512KB DMA per image each way), and
M is sent band-compacted (137KB). PSUM->SBUF de-interleave copies are
split between DVE and Act so neither engine bottlenecks; program order
interleaves stage1(img) with stage2(img-1) so the PE never waits on the
V copies.
"""

import sys

sys.path.insert(0, "/opt/trn_rl_repo")

import numpy as np

L = 512
NJ = L // 2  # 256
TAPS = 8
N_CORES = 8
IMGS_PER_CORE = 12  # 32 batch * 3 channels / 8 cores
BW = 134  # nonzero interleaved-column band width per 128-row chunk

_compiled = {}

# Banded matmul slices per accumulation group: (chunk, src0, src1, dst0, dst1)
# src = cols of the compact band tile, dst = cols of the 512-wide PSUM bank.
# Chunk c covers interleaved cols [128c-6, 128c+128) mod 512; c=0 wraps and
# splits in two. Order keeps the tiny 6-wide stream between long ones so its
# weight load hides behind them.
_SLICES = [
    (0, 0, 6, 506, 512),
    (1, 0, BW, 122, 256),
    (0, 6, BW, 0, 128),
    (2, 0, BW, 250, 384),
    (3, 0, BW, 378, 512),
]


def _build_mc(dec: np.ndarray) -> np.ndarray:
    """Compact banded filter matrix: mc[p, c*134 + k] = M[128c+p, (128c-6+k)%512]
    where M[i, 2j+f] = dec[f][(i-2j) % 512] (zero unless (i-2j)%512 < 8)."""
    M = np.zeros((L, L), dtype=np.float32)
    i = np.arange(L)[:, None]
    j = np.arange(NJ)[None, :]
    k = (i - 2 * j) % L
    mask = k < TAPS
    for f in range(2):
        M[:, f::2] = np.where(mask, np.asarray(dec[f])[np.minimum(k, TAPS - 1)], 0.0)
    mc = np.zeros((128, 4 * BW), dtype=np.float16)
    for c in range(4):
        cols = (128 * c - 6 + np.arange(BW)) % L
        mc[:, BW * c : BW * (c + 1)] = M[128 * c : 128 * c + 128, cols]
    return mc


def _build_nc():
    import concourse.bass as bass  # noqa: F401
    import concourse.tile as tile
    from concourse import bacc, mybir

    f32 = mybir.dt.float32
    f16 = mybir.dt.float16
    nc = bacc.Bacc("TRN2", target_bir_lowering=False, debug=False,
                   num_devices=N_CORES)
    x_d = nc.dram_tensor("xc", [IMGS_PER_CORE, 128, 4 * L], f16,
                         kind="ExternalInput")
    mc_d = nc.dram_tensor("mc", [128, 4 * BW], f16, kind="ExternalInput")
    o_d = nc.dram_tensor("out", [IMGS_PER_CORE, 2, 128, 4 * NJ], f16,
                         kind="ExternalOutput")

    with tile.TileContext(nc) as tc:
        with (
            tc.tile_pool(name="mpool", bufs=1) as mpool,
            tc.tile_pool(name="xpool", bufs=6) as xpool,
            tc.tile_pool(name="vpool", bufs=2) as vpool,
            tc.tile_pool(name="opool", bufs=6) as opool,
            tc.tile_pool(name="pvpool", bufs=2, space="PSUM") as pvpool,
            tc.tile_pool(name="popool", bufs=2, space="PSUM") as popool,
        ):
            mct = mpool.tile([128, 4 * BW], f16, tag="mct")
            # ring the small filter-bank DMA from Act, in parallel with SP
            # ringing image 0's chunks, so neither delays the other
            nc.scalar.dma_start(mct[:], mc_d[:])

            def group(psum_bank, stationary_of_chunk):
                """One 512-col accumulation group: 5 banded matmuls."""
                for n, (c, s0, s1, d0, d1) in enumerate(_SLICES):
                    nc.tensor.matmul(
                        psum_bank[:, d0:d1],
                        stationary_of_chunk(c),
                        mct[:, BW * c + s0 : BW * c + s1],
                        start=(n == 0),
                        stop=(n == len(_SLICES) - 1),
                    )

            vts = [None, None]  # vt of img, img-1

            def stage1(img, xt):
                vt = vpool.tile([128, 4 * L], f16, tag="vt")
                for pair in range(2):
                    pv = pvpool.tile([128, 2 * L], f32, tag="pv")  # 2 banks
                    for wi in range(2):
                        wc = 2 * pair + wi
                        group(
                            pv[:, L * wi : L * wi + L],
                            lambda c: xt[:, L * c + 128 * wc : L * c + 128 * wc + 128],
                        )
                    # de-interleave V: dst (wi, a, j) <- src[512wi + 2j + a];
                    # halves on DVE + Act in parallel so the PSUM pair frees
                    # in ~1us and the PE never stalls (keeps its DVFS ramp)
                    for wi, eng in ((0, nc.vector.tensor_copy), (1, nc.scalar.copy)):
                        eng(
                            vt[:, 2 * L * pair + L * wi : 2 * L * pair + L * (wi + 1)]
                            .rearrange("p (a j) -> p a j", a=2),
                            pv[:, L * wi : L * (wi + 1)].rearrange(
                                "p (j a) -> p a j", j=NJ, a=2
                            ),
                        )
                return vt

            def stage2(img, vt):
                ot = opool.tile([128, 8 * NJ], f16, tag="ot")
                for hjc in range(2):
                    po = popool.tile([128, 2 * L], f32, tag="po")
                    for a in range(2):
                        off = NJ * a + 128 * hjc
                        group(
                            po[:, L * a : L * a + L],
                            lambda c: vt[:, L * c + off : L * c + off + 128],
                        )
                    # subbands: dst (b, a, wj) <- src[p, 512a + 2wj + b];
                    # b-halves split across DVE + Act
                    for b, eng in ((0, nc.vector.tensor_copy), (1, nc.scalar.copy)):
                        eng(
                            ot[:, 4 * NJ * hjc + 2 * NJ * b : 4 * NJ * hjc + 2 * NJ * (b + 1)]
                            .rearrange("p (a w) -> p a w", a=2),
                            po[:, b : 2 * L : 2].rearrange(
                                "p (a w) -> p a w", a=2
                            ),
                        )
                # output rings on SP: Act is near its throughput budget
                # with the copies, and input prefetch has enough runway that
                # head-of-line waits on SP are harmless. The last image rings
                # per-half so its first half transfers while the second
                # half's copy still runs (shorter tail).
                if img == IMGS_PER_CORE - 1:
                    for hjc in range(2):
                        nc.sync.dma_start(
                            o_d[img, hjc],
                            ot[:, 4 * NJ * hjc : 4 * NJ * (hjc + 1)],
                        )
                else:
                    nc.sync.dma_start(
                        o_d[img].rearrange("h p c -> p h c"),
                        ot[:].rearrange("p (h c) -> p h c", h=2),
                    )

            # input rings keep a 4-image lead over output rings in SP's
            # in-order stream: enough lead that a late input never stalls
            # the PE, but outputs aren't pushed behind the whole input chain
            LEAD = 4
            xts = []

            def ring_input(img):
                xt = xpool.tile([128, 4 * L], f16, tag="xt")
                xts.append(xt)
                if img == 0:
                    # split the first image per chunk so its chunk-0
                    # matmuls start ~2us before the full image lands
                    for c in range(4):
                        nc.sync.dma_start(
                            xt[:, L * c : L * (c + 1)],
                            x_d[img][:, L * c : L * (c + 1)],
                        )
                else:
                    nc.sync.dma_start(xt[:], x_d[img])

            for img in range(LEAD):
                ring_input(img)

            # software pipeline: stage1(img) then stage2(img-1), so stage2's
            # dependence on the V copies never stalls the PE.
            for img in range(IMGS_PER_CORE + 1):
                if img + LEAD < IMGS_PER_CORE:
                    ring_input(img + LEAD)
                if img < IMGS_PER_CORE:
                    vts[0] = stage1(img, xts[img])
                if img > 0:
                    stage2(img - 1, vts[1])
                vts[1] = vts[0]

    nc.finalize()
    return nc


def _in_maps(x: np.ndarray, dec: np.ndarray) -> list[dict]:
    mc = _build_mc(dec)
    # host pre-shuffle to (img, p, (c, w)): each image is one contiguous
    # 512KB DMA with 4KB per-partition lines
    xc = np.ascontiguousarray(
        x.reshape(96, 4, 128, L).astype(np.float16).transpose(0, 2, 1, 3)
    ).reshape(96, 128, 4 * L)
    return [
        {"xc": xc[IMGS_PER_CORE * c : IMGS_PER_CORE * (c + 1)], "mc": mc}
        for c in range(N_CORES)
    ]


def kernel(x: np.ndarray, dec: np.ndarray) -> np.ndarray:
    from concourse.bass_utils import run_bass_kernel_spmd

    x = np.ascontiguousarray(np.asarray(x, dtype=np.float32))
    dec = np.asarray(dec, dtype=np.float32)
    B, C, H, W = x.shape
    assert (B, C, H, W) == (32, 3, 512, 512) and dec.shape == (2, 8)

    if "nc" not in _compiled:
        _compiled["nc"] = _build_nc()
    nc = _compiled["nc"]

    in_maps = _in_maps(x, dec)
    res = run_bass_kernel_spmd(nc, in_maps, list(range(N_CORES))).results
    o = np.concatenate([r["out"] for r in res], axis=0)  # (96, 2, 128, 1024) f16
    # o[i, hjc, p, (s, wj)] -> out[i, s, 128*hjc + p, wj]
    o = o.reshape(96, 2, 128, 4, NJ).transpose(0, 3, 1, 2, 4)
    return np.ascontiguousarray(o, dtype=np.float32).reshape(B, C * 4, H // 2, W // 2)
